# revision 3
# baseline (speedup 1.0000x reference)
"""Trainium2 Bass kernel for nn_MultiHeadAttention_84576495993495.

Key observation: the reference module's output einsum is
    out = einsum('bhqk,bhvo->bhvo', attn, v)
which contracts softmax(attn) over BOTH q and k. Every softmax row sums
to 1, so sum_{q,k} attn == S (= 2048) and the whole attention block
collapses to out == S * v. Hence

    reference(x, ...) == ((x @ Wv.T + bv) * S) @ Wp.T + bp
                      ==  x @ M + c
with
    M = S * Wv.T @ Wp.T          (folded on host, fp64 then cast)
    c = S * Wp @ bv + bp

(Verified vs the jax reference: rel Frobenius err ~4.6e-7, i.e. fp32
noise.)  The device work is the data-dependent GEMM x @ M + c, sharded
data-parallel over the 8192 rows: 1024 rows per NeuronCore.

Per core: y_rows = xT.T @ M + c via 8 r-tiles x 2 n-chunks x 8 k-step
PSUM-accumulated fp32 matmuls. The x shard is pre-transposed on the
host (layout prep) because the TensorE contracts over the partition
dim; fp32 has no DMA-transpose path on TRN2.
"""

import os
from functools import lru_cache

import numpy as np

import concourse.bass as bass
import concourse.mybir as mybir
import concourse.tile as tile
from concourse import bacc
from concourse.bass_utils import run_bass_kernel_spmd

N_CORES = 8
P = 128
D = 1024                       # model dim (= SLICE_SIZE)
B, S = 4, 2048
R_TOTAL = B * S                # 8192 rows
R_CORE = R_TOTAL // N_CORES    # 1024 rows per core
K_TILES = D // P               # 8
R_TILES = R_CORE // P          # 8
N_CHUNK = 512                  # fp32 moving-operand max / one PSUM bank
N_CHUNKS = D // N_CHUNK        # 2
SCALE = float(S)               # module divides scores by sqrt(D); softmax sums to 1

# matmul input dtype: float32 (exact, 4 cyc/row) or float32r (1 cyc/row at N>=256)
MM_DTYPE_NAME = os.environ.get("KMM_DTYPE", "float32")


@lru_cache(maxsize=4)
def _build_nc(mm_dtype_name: str, loop_iters: int | None = None):
    """loop_iters: when set, wrap the compute body in a tc.For_i hardware
    loop (inputs loaded once) — used by the benchmark harness to measure
    steady-state per-iteration device time without NTFF profiling."""
    mm_dt = getattr(mybir.dt, mm_dtype_name)
    nc = bacc.Bacc(None, target_bir_lowering=False)

    xT = nc.dram_tensor("xT", [D, R_CORE], mm_dt, kind="ExternalInput")
    Mw = nc.dram_tensor("Mw", [D, D], mm_dt, kind="ExternalInput")
    cb = nc.dram_tensor("cb", [P, D], mybir.dt.float32, kind="ExternalInput")
    y = nc.dram_tensor("y", [R_CORE, D], mybir.dt.float32, kind="ExternalOutput")

    xT_t = xT.rearrange("(ko p) r -> p ko r", p=P)   # [128, 8, 1024]
    Mw_t = Mw.rearrange("(ko p) n -> p ko n", p=P)   # [128, 8, 1024]

    with tile.TileContext(nc) as tc:
        with (
            tc.tile_pool(name="wpool", bufs=1) as wpool,
            tc.tile_pool(name="opool", bufs=4) as opool,
            tc.tile_pool(name="pspool", bufs=4, space="PSUM") as pspool,
        ):
            xT_sb = wpool.tile([P, K_TILES, R_CORE], mm_dt, tag="xT_sb")
            M_sb = wpool.tile([P, K_TILES, D], mm_dt, tag="M_sb")
            cb_sb = wpool.tile([P, D], mybir.dt.float32, tag="cb_sb")

            nc.sync.dma_start(cb_sb[:], cb[:])
            for k in range(K_TILES):
                nc.sync.dma_start(xT_sb[:, k], xT_t[:, k])
                nc.sync.dma_start(M_sb[:, k], Mw_t[:, k])

            def body():
                for r in range(R_TILES):
                    for nch in range(N_CHUNKS):
                        ps = pspool.tile(
                            [P, N_CHUNK], mybir.dt.float32, tag="ps"
                        )
                        for k in range(K_TILES):
                            nc.tensor.matmul(
                                ps[:],
                                xT_sb[:, k, bass.ts(r, P)],
                                M_sb[:, k, bass.ts(nch, N_CHUNK)],
                                start=(k == 0),
                                stop=(k == K_TILES - 1),
                            )
                        out_sb = opool.tile(
                            [P, N_CHUNK], mybir.dt.float32, tag="out_sb"
                        )
                        nc.vector.tensor_add(
                            out_sb[:], ps[:], cb_sb[:, bass.ts(nch, N_CHUNK)]
                        )
                        nc.sync.dma_start(
                            y[bass.ts(r, P), bass.ts(nch, N_CHUNK)], out_sb[:]
                        )

            if loop_iters is None:
                body()
            else:
                with tc.For_i(0, loop_iters, 1):
                    body()
    nc.compile()
    return nc


def _host_prep(x, Wv, bv, Wp, bp):
    X = np.ascontiguousarray(x, dtype=np.float32).reshape(R_TOTAL, D)
    M = (SCALE * (Wv.T.astype(np.float64) @ Wp.T.astype(np.float64))).astype(
        np.float32
    )
    c = (SCALE * (Wp.astype(np.float64) @ bv.astype(np.float64)) + bp).astype(
        np.float32
    )
    cbt = np.ascontiguousarray(np.broadcast_to(c, (P, D)))
    in_maps = []
    for i in range(N_CORES):
        shard = X[i * R_CORE : (i + 1) * R_CORE]
        in_maps.append(
            {
                "xT": np.ascontiguousarray(shard.T),
                "Mw": M,
                "cb": cbt,
            }
        )
    return in_maps


def kernel(x, Wq, bq, Wk, bk, Wv, bv, Wp, bp):
    nc = _build_nc(MM_DTYPE_NAME)
    in_maps = _host_prep(x, Wv, bv, Wp, bp)
    res = run_bass_kernel_spmd(nc, in_maps, core_ids=list(range(N_CORES)))
    y = np.concatenate([r["y"] for r in res.results], axis=0)
    return y.reshape(B, S, D)
